# revision 19
# baseline (speedup 1.0000x reference)
"""Trainium2 Bass kernel for multi-query attention with tanh-clamped softmax.

Sharding: each core takes ONE batch and TWO query heads (8 cores = 2 batches x
4 head-pairs). K/V projections are shared (multi-query) so they are replicated
across the 4 cores of a batch; Q columns and W_out rows are tensor-parallel by
head. Each core emits per-head partial outputs [2, n, dim]; the host sums the
8 partial planes per batch (row-parallel unshard).

Versus the 1-head x 2-batch layout this halves the projected token count
(stage A: 72->46us PE) and halves the number of host partials.

All matmuls run in bf16 with fp32 PSUM accumulation. Key scheduling choices:
- The attn bias is added on the DVE during the sim PSUM->SBUF evacuation
  (tensor_tensor add), NOT via an identity matmul on the PE: saves ~27us of
  PE time, and the SBUF staging lets tanh/exp run as wide [128,4096] ACT
  instructions (4 key-tiles per instr), cutting ACT fixed overheads.
- LN rsqrt runs as a DVE-only Newton iteration everywhere (the ACT Sqrt
  would thrash ~1.3us activation-table reloads against tanh/exp).
- Row tiles 8-15 of the QKV projection are injected into the first
  attention pass's kt-loop so the PE queue stays dense; their x chunks are
  prefetched on the second HWDGE queue.
- Softmax row-sums ride a ones-column in the attn@v accumulation; their
  reciprocal is reshaped to per-partition columns via a DRAM bounce and
  folded into the per-head output-projection evacuation as a per-token scale.
- ~3.5us of dummy matmuls pre-warm the PE HAM clock gate during the DMA head.
"""

import os
import sys

sys.path.insert(0, "/opt/trn_rl_repo")

import numpy as np
import ml_dtypes

import concourse.bass as bass
import concourse.tile as tile
from concourse import bacc, mybir
from concourse.bass_utils import run_bass_kernel_spmd
from concourse.masks import make_identity

F32 = mybir.dt.float32
BF16 = mybir.dt.bfloat16
AF = mybir.ActivationFunctionType
ALU = mybir.AluOpType

HEADS = 8
DQK = 128
DV = 192
SCALE = 64 ** -0.5
CLAMP = 5.0
EPS = 1e-5

B = 2
N = 2048
DIM = 1536
N_CORES = 8
HPC = 2                      # heads per core

_LAST_STATS = {}


def _enable_ldw_opt():
    """The container's baked compiler flags carry --enable-ldw-opt=false;
    flip just that option for this kernel's compile (it removes the
    per-matmul serialized LDWEIGHTS)."""
    try:
        from concourse.compiler_utils import get_compiler_flags, set_compiler_flags

        flags = [f.replace("--enable-ldw-opt=false", "--enable-ldw-opt=true")
                 for f in get_compiler_flags()]
        set_compiler_flags(flags)
    except Exception:
        pass


def build_nc(n=N, dim=DIM):
    """Build the per-core Bass graph. All cores run the same graph (SPMD)."""
    _enable_ldw_opt()
    assert dim % 512 == 0 and n % 1024 == 0
    DIMT = dim // 128          # 12 contraction tiles for projections
    RT = n // 128              # 16 row tiles (one batch)
    KT = n // 128              # 16 key tiles
    QH = n // 1024             # 2 q-half passes
    QHW = 1024
    QC = QHW // 512            # 2
    CC = dim // 512            # 3 output column chunks
    WCOLS = HPC * DQK + DQK + DV   # 576

    nc = bacc.Bacc("TRN2", target_bir_lowering=False)

    xT = nc.declare_dram_parameter("xT", [dim, n], BF16, isOutput=False)
    w_all = nc.declare_dram_parameter("w_all", [dim, WCOLS], BF16, isOutput=False)
    biasT = nc.declare_dram_parameter("biasT", [HPC, n, n], BF16, isOutput=False)
    w_out = nc.declare_dram_parameter("w_out", [HPC * DV, dim], BF16, isOutput=False)
    gq = nc.declare_dram_parameter("gq", [DQK, 1], F32, isOutput=False)
    gk = nc.declare_dram_parameter("gk", [DQK, 1], F32, isOutput=False)
    out = nc.declare_dram_parameter("out", [HPC, n, dim], BF16, isOutput=True)

    with tile.TileContext(nc) as tc:
        with (
            tc.tile_pool(name="const", bufs=1) as const,
            tc.tile_pool(name="big", bufs=1) as big,
            tc.tile_pool(name="stA", bufs=5) as sA,
            tc.tile_pool(name="stB", bufs=3) as sB,
            tc.tile_pool(name="biasp", bufs=4) as sBias,
            tc.tile_pool(name="thp", bufs=2) as sTh,
            tc.tile_pool(name="t4p", bufs=1) as sT4,
            tc.tile_pool(name="expp", bufs=2) as sE,
            tc.tile_pool(name="dramp", bufs=3, space="DRAM") as sDram,
            tc.tile_pool(name="work_ps", bufs=4, space="PSUM") as psW,
            tc.tile_pool(name="acc_ps", bufs=1, space="PSUM") as psAcc,
        ):
            # ---------------- constants ----------------
            DTH = DIMT // 2
            w_all_sb = const.tile([128, DIMT, WCOLS], BF16)
            w_all_r = w_all.rearrange("(t p) c -> p t c", p=128)
            nc.sync.dma_start(out=w_all_sb[:, :DTH, :], in_=w_all_r[:, :DTH, :])
            nc.sync.dma_start(out=w_all_sb[:, DTH:, :], in_=w_all_r[:, DTH:, :])
            gq_sb = const.tile([128, 1], F32)
            nc.sync.dma_start(out=gq_sb, in_=gq[:, :])
            gk_sb = const.tile([128, 1], F32)
            nc.sync.dma_start(out=gk_sb, in_=gk[:, :])
            ident = const.tile([128, 128], BF16)
            make_identity(nc, ident)
            I32 = mybir.dt.int32
            magic_sb = const.tile([128, 2, 4], I32)
            nc.vector.memset(magic_sb, 0x5F3759DF)
            # HAM pre-warm: dummy PE activity during the DMA head so the
            # first real matmuls start at the full clock
            warm_sb = const.tile([128, 128], BF16)
            nc.vector.memset(warm_sb, 0.0)
            warm_ps = psW.tile([128, 512], F32, name="warm_ps", tag="w", bufs=2)
            for _ in range(52):
                nc.tensor.matmul(warm_ps[:, :128], lhsT=warm_sb, rhs=warm_sb,
                                 start=True, stop=True)

            # ---------------- resident activations ----------------
            NXC = 4
            XCW = n // NXC                 # 512 tokens per x chunk
            RT_PER_XC = XCW // 128         # 4 row tiles per chunk
            xTr = xT.rearrange("(t p) r -> p t r", p=128)

            qT_sb = [big.tile([128, n], BF16, name=f"qT{h}") for h in range(HPC)]
            kT_sb = big.tile([128, n], BF16, name="kT")
            v_sb = big.tile([128, KT, 208], BF16, name="v")
            nc.vector.memset(v_sb[:, :, DV:DV + 1], 1.0)

            # ---------------- stage A: QKV projection + LN + transpose ----------
            pending_tr = []

            def emit_tr(ktile_, qn0_, qn1_, kn_):
                dsts = [(qn0_, qT_sb[0], gq_sb), (qn1_, qT_sb[1], gq_sb),
                        (kn_, kT_sb, gk_sb)]
                for src, dstt, g in dsts:
                    tp = psW.tile([128, 512], BF16, name="tp", tag="w", bufs=2)[:, :128]
                    nc.tensor.transpose(tp, src, ident)
                    nc.vector.tensor_scalar_mul(
                        out=dstt[:, ktile_ * 128:(ktile_ + 1) * 128],
                        in0=tp, scalar1=g)

            _rt_state = {"mvp": None, "held": None}
            _xt_chunks = {}

            def load_xt(xc, engine=None):
                eng = engine or nc.sync
                xt_sb = sA.tile([128, DIMT, XCW], BF16, name="xt_sb", tag="xt",
                                bufs=2)
                nsplit = 4 if xc == 0 else 2
                step = DIMT // nsplit
                for s in range(nsplit):
                    eng.dma_start(
                        out=xt_sb[:, s * step:(s + 1) * step, :],
                        in_=xTr[:, s * step:(s + 1) * step,
                                xc * XCW:(xc + 1) * XCW],
                    )
                _xt_chunks[xc] = xt_sb

            # segments of the qkv projection output: q_h0, q_h1, k, v
            segs = [(0, DQK), (DQK, DQK), (2 * DQK, DQK), (3 * DQK, DV)]

            def emit_rt(rt):
                xc = rt // RT_PER_XC
                xoff = (rt % RT_PER_XC) * 128
                if xc not in _xt_chunks:
                    load_xt(xc)
                xt_sb = _xt_chunks[xc]
                # ready transposes fill the PE while the qkv psum ring waits
                # on the previous row tile's evacuation
                for _ in range(2):
                    if len(pending_tr) >= 2:
                        emit_tr(*pending_tr.pop(0))

                # 576 psum columns span 2 banks -> two matmul targets
                qkv_a = psW.tile([128, 512], F32, name="qkv_a", tag="w", bufs=2)
                qkv_b = psW.tile([128, 512], F32, name="qkv_b", tag="w", bufs=2)[:, :64]
                for dt_ in range(DIMT):
                    nc.tensor.matmul(
                        qkv_a, lhsT=xt_sb[:, dt_, xoff:xoff + 128],
                        rhs=w_all_sb[:, dt_, 0:512],
                        start=(dt_ == 0), stop=(dt_ == DIMT - 1))
                    nc.tensor.matmul(
                        qkv_b, lhsT=xt_sb[:, dt_, xoff:xoff + 128],
                        rhs=w_all_sb[:, dt_, 512:WCOLS],
                        start=(dt_ == 0), stop=(dt_ == DIMT - 1))
                qkv_sb = sA.tile([128, WCOLS], F32, name="qkv_sb")
                nc.scalar.activation(out=qkv_sb[:, 0:512], in_=qkv_a,
                                     func=AF.Copy)
                nc.scalar.activation(out=qkv_sb[:, 512:WCOLS], in_=qkv_b,
                                     func=AF.Copy)

                # layernorm stats; rt pairs share one DVE Newton rsqrt
                par = rt % 2
                if par == 0:
                    _rt_state["mvp"] = sA.tile([128, 2, 4, 2], F32, name="mvp")
                mvp = _rt_state["mvp"]
                stats = sA.tile([128, 4, 6], F32, name="stats")
                for si, (off, w) in enumerate(segs):
                    nc.vector.bn_stats(out=stats[:, si, :], in_=qkv_sb[:, off:off + w])
                    nc.vector.bn_aggr(out=mvp[:, par, si, :], in_=stats[:, si, :])
                if par == 0:
                    _rt_state["held"] = (rt, qkv_sb)
                    return
                held_rt, held_sb = _rt_state["held"]
                # DVE-only Newton rsqrt of (var + EPS)
                rstd = sA.tile([128, 2, 4], F32, name="rstd")
                xv = sA.tile([128, 2, 4], F32, name="rsq_x")
                nc.vector.tensor_scalar_add(out=xv, in0=mvp[:, :, :, 1],
                                            scalar1=EPS)
                yi = sA.tile([128, 2, 4], I32, name="rsq_yi")
                nc.vector.tensor_scalar(
                    out=yi, in0=xv.bitcast(I32), scalar1=1, scalar2=None,
                    op0=ALU.logical_shift_right)
                nc.vector.tensor_tensor(out=yi, in0=magic_sb, in1=yi,
                                        op=ALU.subtract)
                y = yi.bitcast(F32)
                h_ = sA.tile([128, 2, 4], F32, name="rsq_h")
                for _ in range(3):
                    nc.vector.tensor_tensor(out=h_, in0=y, in1=y, op=ALU.mult)
                    nc.vector.tensor_tensor(out=h_, in0=h_, in1=xv, op=ALU.mult)
                    nc.vector.tensor_scalar(
                        out=h_, in0=h_, scalar1=-0.5, scalar2=1.5,
                        op0=ALU.mult, op1=ALU.add)
                    nc.vector.tensor_tensor(out=y, in0=y, in1=h_, op=ALU.mult)
                nc.vector.tensor_copy(out=rstd, in_=y)
                nmr = sA.tile([128, 2, 4], F32, name="nmr")
                nc.vector.tensor_tensor(out=nmr, in0=mvp[:, :, :, 0],
                                        in1=rstd, op=ALU.mult)
                nc.vector.tensor_scalar_mul(out=nmr, in0=nmr, scalar1=-1.0)

                for pp, (rt_, sb_) in enumerate([(held_rt, held_sb),
                                                 (rt, qkv_sb)]):
                    norm = []
                    for si, (off, w) in enumerate(segs):
                        if si < 3:
                            dst = sA.tile([128, 128], BF16, name=f"n{si}")
                        else:
                            dst = v_sb[:, rt_, 0:DV]
                        if si < 2:
                            nc.vector.tensor_scalar(
                                out=dst, in0=sb_[:, off:off + w],
                                scalar1=mvp[:, pp, si, 0:1],
                                scalar2=rstd[:, pp, si:si + 1],
                                op0=ALU.subtract, op1=ALU.mult)
                        else:
                            nc.scalar.activation(
                                out=dst, in_=sb_[:, off:off + w],
                                func=AF.Identity,
                                scale=rstd[:, pp, si:si + 1],
                                bias=nmr[:, pp, si:si + 1])
                        if si < 3:
                            norm.append(dst)
                    pending_tr.append((rt_, norm[0], norm[1], norm[2]))

            def flush_tr():
                while pending_tr:
                    emit_tr(*pending_tr.pop(0))

            # phase 1: row tiles 0-7 stand alone (needed before any attention);
            # row tiles 8-15 are injected into the first attention kt-loop.
            for rt in range(8):
                emit_rt(rt)
            flush_tr()
            load_xt(2, engine=nc.scalar)
            load_xt(3, engine=nc.scalar)
            inject_rts = list(range(8, RT))

            # ---------------- stage B: attention + output projection --------
            # w_out loaded here so the head DMA queue serves stage A first.
            w_out_t = []
            for h in range(HPC):
                wa = const.tile([128, dim], BF16)
                nc.scalar.dma_start(out=wa, in_=w_out[h * DV:h * DV + 128, :])
                wb = const.tile([64, dim], BF16)
                nc.scalar.dma_start(out=wb, in_=w_out[h * DV + 128:(h + 1) * DV, :])
                w_out_t.append((wa, wb))

            import functools

            pending_po = []

            def emit_po(outUa_, outUb_, rcol_, h_, qoff_, t, cc, tail=False):
                po = psW.tile([128, 512], F32, name="po", tag="w", bufs=2)
                wa, wb = w_out_t[h_]
                nc.tensor.matmul(
                    po, lhsT=outUa_[:, t * 128:(t + 1) * 128],
                    rhs=wa[:, cc * 512:(cc + 1) * 512],
                    start=True, stop=False)
                nc.tensor.matmul(
                    po, lhsT=outUb_[:, t * 128:(t + 1) * 128],
                    rhs=wb[:, cc * 512:(cc + 1) * 512],
                    start=False, stop=True)
                po_sb = sB.tile([128, 512], BF16, name="po_sb")
                if (t * CC + cc) % 2 == 1:
                    nc.scalar.activation(out=po_sb, in_=po, func=AF.Copy,
                                         scale=rcol_[:, t:t + 1])
                else:
                    nc.vector.tensor_scalar_mul(
                        out=po_sb, in0=po, scalar1=rcol_[:, t:t + 1])
                eng = nc.scalar if (tail and (t * CC + cc) % 2 == 0) else nc.sync
                eng.dma_start(
                    out=out[h_, qoff_ + t * 128: qoff_ + (t + 1) * 128,
                            cc * 512:(cc + 1) * 512],
                    in_=po_sb)

            for h in range(HPC):
                for qh in range(QH):
                    qoff = qh * QHW
                    accA = [psAcc.tile([128, 512], F32, name=f"accA{qc}",
                                       tag=f"accA{qc}")
                            for qc in range(QC)]
                    accB = [psAcc.tile([65, 512], F32, name=f"accB{qc}",
                                       tag=f"accB{qc}")
                            for qc in range(QC)]

                    def fire_av(pkt, e1):
                        # e1: [128, QHW] exp row for key tile pkt
                        for qc in range(QC):
                            nc.tensor.matmul(
                                accA[qc], lhsT=v_sb[:, pkt, 0:128],
                                rhs=e1[:, qc * 512:(qc + 1) * 512],
                                start=(pkt == 0), stop=(pkt == KT - 1))
                            nc.tensor.matmul(
                                accB[qc], lhsT=v_sb[:, pkt, 128:DV + 1],
                                rhs=e1[:, qc * 512:(qc + 1) * 512],
                                start=(pkt == 0), stop=(pkt == KT - 1))

                    pending_av = []
                    for kt in range(KT):
                        # exp group sizes [2,2,4,4,2,1,1]: short groups at the
                        # pass edges cut the sim->tanh->exp->av latency bubble
                        if kt < 4 or kt in (12, 13):
                            gsz, gslot = 2, kt % 2
                        elif kt >= 14:
                            gsz, gslot = 1, 0
                        else:
                            gsz, gslot = 4, (kt - 4) % 4
                        if gslot == 0:
                            t4 = sT4.tile([128, gsz, QHW], BF16, name="t4",
                                          bufs=2)
                        bias_sb = sBias.tile([128, QHW], BF16, name="bias_sb")
                        nc.sync.dma_start(
                            out=bias_sb,
                            in_=biasT[h, kt * 128:(kt + 1) * 128,
                                      qoff:qoff + QHW])

                        # --- older, dependency-free work first so the PE
                        # FIFO never head-of-line blocks on this kt's sim ---
                        nfire = (min(3, len(pending_av)) if kt >= KT - 2
                                 else (2 if len(pending_av) >= 3 else 0))
                        for _ in range(nfire):
                            fire_av(*pending_av.pop(0))

                        if kt >= 2:
                            for _ in range(2):
                                if pending_po:
                                    pending_po.pop(0)()

                        if h == 0 and qh == 0:
                            if inject_rts:
                                emit_rt(inject_rts.pop(0))
                            elif pending_tr:
                                # transposes of injected rts must land before
                                # their kT columns are consumed in THIS loop
                                emit_tr(*pending_tr.pop(0))
                                if kt >= 10:
                                    flush_tr()

                        # both q-chunks of this kt share one 2-bank psum tile
                        # so ONE [128,1024] tanh evacuates it; the bias rides
                        # an identity matmul into the accumulation (zero
                        # PSUM-read traffic -- concurrent DVE/ACT psum reads
                        # throttle the PE's drain)
                        sim_ps = psW.tile([128, QC, 512], F32, name="sim_ps",
                                          tag="sim", bufs=1)
                        for qc in range(QC):
                            nc.tensor.matmul(
                                sim_ps[:, qc, :],
                                lhsT=kT_sb[:, kt * 128:(kt + 1) * 128],
                                rhs=qT_sb[h][:, qoff + qc * 512:
                                             qoff + (qc + 1) * 512],
                                start=True, stop=False)
                            nc.tensor.matmul(
                                sim_ps[:, qc, :],
                                lhsT=ident,
                                rhs=bias_sb[:, qc * 512:(qc + 1) * 512],
                                start=False, stop=True)
                        nc.scalar.activation(out=t4[:, gslot, :], in_=sim_ps,
                                             func=AF.Tanh, scale=1.0 / CLAMP)
                        if gslot == gsz - 1:
                            e4 = sE.tile([128, gsz, QHW], BF16, name="e4")
                            nc.scalar.activation(out=e4, in_=t4, func=AF.Exp,
                                                 scale=CLAMP)
                            for j in range(gsz):
                                pending_av.append((kt - gsz + 1 + j,
                                                   e4[:, j, :]))

                    for pkt, e1 in pending_av:
                        fire_av(pkt, e1)

                    def make_evac(accA_, accB_, h_, qoff_):
                        def evac():
                            outUa = sB.tile([128, QHW], BF16, name="outUa")
                            outUb = sB.tile([64, QHW], BF16, name="outUb")
                            s_row = sB.tile([1, QHW], F32, name="s_row")
                            for qc in range(QC):
                                nc.vector.tensor_copy(
                                    out=s_row[:, qc * 512:(qc + 1) * 512],
                                    in_=accB_[qc][64:65, :])
                            s_dram = sDram.tile([1, QHW], F32, name="s_dram")
                            nc.sync.dma_start(out=s_dram, in_=s_row)
                            for qc in range(QC):
                                # ACT copies: they land in ACT's idle window
                                # between the last exp and the next pass tanh
                                nc.scalar.activation(
                                    out=outUa[:, qc * 512:(qc + 1) * 512],
                                    in_=accA_[qc], func=AF.Copy)
                                nc.scalar.activation(
                                    out=outUb[:, qc * 512:(qc + 1) * 512],
                                    in_=accB_[qc][0:64, :], func=AF.Copy)
                            rcol_raw = sB.tile([128, QHW // 128], F32,
                                               name="rcol_raw")
                            nc.sync.dma_start(
                                out=rcol_raw,
                                in_=s_dram.rearrange("one (t p) -> p (one t)",
                                                     p=128))
                            rcol = sB.tile([128, QHW // 128], F32, name="rcol")
                            nc.vector.reciprocal(out=rcol, in_=rcol_raw)
                            for t in range(QHW // 128):
                                for cc in range(CC):
                                    pending_po.append(functools.partial(
                                        emit_po, outUa, outUb, rcol,
                                        h_, qoff_, t, cc))
                        return evac

                    make_evac(accA, accB, h, qoff)()

                if h == 0:
                    flush_tr()

            for fn in pending_po:
                fn(tail=True)

    nc.compile()
    return nc


_NC_CACHE = {}


def _get_nc(n=N, dim=DIM):
    key = (n, dim)
    if key not in _NC_CACHE:
        _NC_CACHE[key] = build_nc(n, dim)
    return _NC_CACHE[key]


def make_in_maps(x, attn_bias, w_qkv, w_out, g_q, g_k, g_v, n_cores=N_CORES):
    """Host-side shard + preprocess. Returns per-core input maps."""
    b, n, dim = x.shape
    bf = ml_dtypes.bfloat16
    xT_b = [np.ascontiguousarray(x[bb].T).astype(bf) for bb in range(b)]
    kv_cols = w_qkv[:, HEADS * DQK:].astype(np.float32)      # [dim, 320]
    w_out_g = (w_out.astype(np.float32)
               * np.tile(g_v.astype(np.float32), HEADS)[:, None])
    in_maps = []
    for c in range(n_cores):
        bb = c % 2
        h0 = (c // 2) * HPC
        w_q = [w_qkv[:, (h0 + i) * DQK:(h0 + i + 1) * DQK] for i in range(HPC)]
        w_all = np.concatenate(w_q + [kv_cols], axis=1).astype(bf)  # [dim, 576]
        biasT = np.ascontiguousarray(
            attn_bias[bb, h0:h0 + HPC].transpose(0, 2, 1)).astype(bf)
        w_out_h = w_out_g[h0 * DV:(h0 + HPC) * DV, :].astype(bf)   # [384, dim]
        in_maps.append({
            "xT": xT_b[bb],
            "w_all": w_all,
            "biasT": biasT,
            "w_out": w_out_h,
            "gq": (g_q * SCALE).astype(np.float32).reshape(DQK, 1),
            "gk": g_k.astype(np.float32).reshape(DQK, 1),
        })
    return in_maps


def kernel(x, attn_bias, w_qkv, w_out, g_q, g_k, g_v):
    x = np.asarray(x, dtype=np.float32)
    attn_bias = np.asarray(attn_bias, dtype=np.float32)
    w_qkv = np.asarray(w_qkv, dtype=np.float32)
    w_out = np.asarray(w_out, dtype=np.float32)
    g_q = np.asarray(g_q, dtype=np.float32)
    g_k = np.asarray(g_k, dtype=np.float32)
    g_v = np.asarray(g_v, dtype=np.float32)

    b, n, dim = x.shape
    nc = _get_nc(n, dim)
    in_maps = make_in_maps(x, attn_bias, w_qkv, w_out, g_q, g_k, g_v)
    res = run_bass_kernel_spmd(nc, in_maps, core_ids=list(range(N_CORES)),
                               trace=bool(os.environ.get("KERNEL_TRACE")))
    _LAST_STATS["exec_time_ns"] = res.exec_time_ns
    _LAST_STATS["mean_exec_time_ns"] = res.mean_exec_time_ns
    _LAST_STATS["res"] = res
    out = np.zeros((b, n, dim), dtype=np.float32)
    for c in range(N_CORES):
        part = res.results[c]["out"].astype(np.float32)
        out[c % 2] += part[0] + part[1]
    return out


# revision 21
# speedup vs baseline: 1.0235x; 1.0235x over previous
"""Trainium2 Bass kernel for multi-query attention with tanh-clamped softmax.

Sharding: each core takes ONE batch and TWO query heads (8 cores = 2 batches x
4 head-pairs). K/V projections are shared (multi-query) so they are replicated
across the 4 cores of a batch; Q columns and W_out rows are tensor-parallel by
head. Each core emits per-head partial outputs [2, n, dim]; the host sums the
8 partial planes per batch (row-parallel unshard).

Versus the 1-head x 2-batch layout this halves the projected token count
(stage A: 72->46us PE) and halves the number of host partials.

All matmuls run in bf16 with fp32 PSUM accumulation. Key scheduling choices:
- The attn bias is added on the DVE during the sim PSUM->SBUF evacuation
  (tensor_tensor add), NOT via an identity matmul on the PE: saves ~27us of
  PE time, and the SBUF staging lets tanh/exp run as wide [128,4096] ACT
  instructions (4 key-tiles per instr), cutting ACT fixed overheads.
- LN rsqrt runs as a DVE-only Newton iteration everywhere (the ACT Sqrt
  would thrash ~1.3us activation-table reloads against tanh/exp).
- Row tiles 8-15 of the QKV projection are injected into the first
  attention pass's kt-loop so the PE queue stays dense; their x chunks are
  prefetched on the second HWDGE queue.
- Softmax row-sums ride a ones-column in the attn@v accumulation; their
  reciprocal is reshaped to per-partition columns via a DRAM bounce and
  folded into the per-head output-projection evacuation as a per-token scale.
- ~3.5us of dummy matmuls pre-warm the PE HAM clock gate during the DMA head.
"""

import os
import sys

sys.path.insert(0, "/opt/trn_rl_repo")

import numpy as np
import ml_dtypes

import concourse.bass as bass
import concourse.tile as tile
from concourse import bacc, mybir
from concourse.bass_utils import run_bass_kernel_spmd
from concourse.masks import make_identity

F32 = mybir.dt.float32
BF16 = mybir.dt.bfloat16
AF = mybir.ActivationFunctionType
ALU = mybir.AluOpType

HEADS = 8
DQK = 128
DV = 192
SCALE = 64 ** -0.5
CLAMP = 5.0
EPS = 1e-5

B = 2
N = 2048
DIM = 1536
N_CORES = 8
HPC = 2                      # heads per core

_LAST_STATS = {}


def _enable_ldw_opt():
    """The container's baked compiler flags carry --enable-ldw-opt=false;
    flip just that option for this kernel's compile (it removes the
    per-matmul serialized LDWEIGHTS)."""
    try:
        from concourse.compiler_utils import get_compiler_flags, set_compiler_flags

        flags = [f.replace("--enable-ldw-opt=false", "--enable-ldw-opt=true")
                 for f in get_compiler_flags()]
        set_compiler_flags(flags)
    except Exception:
        pass


def build_nc(n=N, dim=DIM):
    """Build the per-core Bass graph. All cores run the same graph (SPMD)."""
    _enable_ldw_opt()
    assert dim % 512 == 0 and n % 1024 == 0
    DIMT = dim // 128          # 12 contraction tiles for projections
    RT = n // 128              # 16 row tiles (one batch)
    KT = n // 128              # 16 key tiles
    QH = n // 1024             # 2 q-half passes
    QHW = 1024
    QC = QHW // 512            # 2
    CC = dim // 512            # 3 output column chunks
    WCOLS = HPC * DQK + DQK + DV   # 576

    nc = bacc.Bacc("TRN2", target_bir_lowering=False)

    xT = nc.declare_dram_parameter("xT", [dim, n], BF16, isOutput=False)
    w_all = nc.declare_dram_parameter("w_all", [dim, WCOLS], BF16, isOutput=False)
    biasT = nc.declare_dram_parameter("biasT", [HPC, n, n], BF16, isOutput=False)
    w_out = nc.declare_dram_parameter("w_out", [HPC * DV, dim], BF16, isOutput=False)
    gq = nc.declare_dram_parameter("gq", [DQK, 1], F32, isOutput=False)
    gk = nc.declare_dram_parameter("gk", [DQK, 1], F32, isOutput=False)
    out = nc.declare_dram_parameter("out", [HPC, n, dim], BF16, isOutput=True)

    with tile.TileContext(nc) as tc:
        with (
            tc.tile_pool(name="const", bufs=1) as const,
            tc.tile_pool(name="big", bufs=1) as big,
            tc.tile_pool(name="stA", bufs=5) as sA,
            tc.tile_pool(name="stB", bufs=3) as sB,
            tc.tile_pool(name="biasp", bufs=6) as sBias,
            tc.tile_pool(name="thp", bufs=2) as sTh,
            tc.tile_pool(name="t4p", bufs=1) as sT4,
            tc.tile_pool(name="expp", bufs=2) as sE,
            tc.tile_pool(name="dramp", bufs=3, space="DRAM") as sDram,
            tc.tile_pool(name="work_ps", bufs=4, space="PSUM") as psW,
            tc.tile_pool(name="acc_ps", bufs=1, space="PSUM") as psAcc,
        ):
            # ---------------- constants ----------------
            DTH = DIMT // 2
            w_all_sb = const.tile([128, DIMT, WCOLS], BF16)
            w_all_r = w_all.rearrange("(t p) c -> p t c", p=128)
            nc.sync.dma_start(out=w_all_sb[:, :DTH, :], in_=w_all_r[:, :DTH, :])
            gq_sb = const.tile([128, 1], F32)
            gk_sb = const.tile([128, 1], F32)
            ident = const.tile([128, 128], BF16)
            make_identity(nc, ident)
            I32 = mybir.dt.int32
            magic_sb = const.tile([128, 4, 4], I32)
            nc.vector.memset(magic_sb, 0x5F3759DF)
            # HAM pre-warm: dummy PE activity during the DMA head so the
            # first real matmuls start at the full clock
            warm_sb = const.tile([128, 128], BF16)
            nc.vector.memset(warm_sb, 0.0)
            warm_ps = psW.tile([128, 512], F32, name="warm_ps", tag="w", bufs=2)
            for _ in range(64):
                nc.tensor.matmul(warm_ps[:, :128], lhsT=warm_sb, rhs=warm_sb,
                                 start=True, stop=True)

            # ---------------- resident activations ----------------
            NXC = 4
            XCW = n // NXC                 # 512 tokens per x chunk
            RT_PER_XC = XCW // 128         # 4 row tiles per chunk
            xTr = xT.rearrange("(t p) r -> p t r", p=128)

            qT_sb = [big.tile([128, n], BF16, name=f"qT{h}") for h in range(HPC)]
            kT_sb = big.tile([128, n], BF16, name="kT")
            v_sb = big.tile([128, KT, 208], BF16, name="v")
            nc.vector.memset(v_sb[:, :, DV:DV + 1], 1.0)

            # ---------------- stage A: QKV projection + LN + transpose ----------
            pending_tr = []

            def emit_tr(ktile_, qn0_, qn1_, kn_):
                dsts = [(qn0_, qT_sb[0], gq_sb), (qn1_, qT_sb[1], gq_sb),
                        (kn_, kT_sb, gk_sb)]
                for src, dstt, g in dsts:
                    tp = psW.tile([128, 512], BF16, name="tp", tag="w", bufs=2)[:, :128]
                    nc.tensor.transpose(tp, src, ident)
                    nc.vector.tensor_scalar_mul(
                        out=dstt[:, ktile_ * 128:(ktile_ + 1) * 128],
                        in0=tp, scalar1=g)

            _grp = {"mvp": None, "held": []}
            _xt_chunks = {}

            def load_xt(xc, engine=None):
                eng = engine or nc.sync
                xt_sb = sA.tile([128, DIMT, XCW], BF16, name="xt_sb", tag="xt",
                                bufs=2)
                nsplit = 4 if xc == 0 else 2
                step = DIMT // nsplit
                for s in range(nsplit):
                    eng.dma_start(
                        out=xt_sb[:, s * step:(s + 1) * step, :],
                        in_=xTr[:, s * step:(s + 1) * step,
                                xc * XCW:(xc + 1) * XCW],
                    )
                _xt_chunks[xc] = xt_sb

            # segments of the qkv projection output: q_h0, q_h1, k, v
            segs = [(0, DQK), (DQK, DQK), (2 * DQK, DQK), (3 * DQK, DV)]

            def finalize_group():
                # one Newton rsqrt chain for FOUR row tiles: the chain is
                # ~20 serial DVE ops, so batching amortizes its latency
                mvp = _grp["mvp"]
                held = _grp["held"]
                _grp["held"] = []
                rstd = sA.tile([128, 4, 4], F32, name="rstd")
                xv = sA.tile([128, 4, 4], F32, name="rsq_x")
                nc.vector.tensor_scalar_add(out=xv, in0=mvp[:, :, :, 1],
                                            scalar1=EPS)
                yi = sA.tile([128, 4, 4], I32, name="rsq_yi")
                nc.vector.tensor_scalar(
                    out=yi, in0=xv.bitcast(I32), scalar1=1, scalar2=None,
                    op0=ALU.logical_shift_right)
                nc.vector.tensor_tensor(out=yi, in0=magic_sb, in1=yi,
                                        op=ALU.subtract)
                y = yi.bitcast(F32)
                h_ = sA.tile([128, 4, 4], F32, name="rsq_h")
                for _ in range(3):
                    nc.vector.tensor_tensor(out=h_, in0=y, in1=y, op=ALU.mult)
                    nc.vector.tensor_tensor(out=h_, in0=h_, in1=xv, op=ALU.mult)
                    nc.vector.tensor_scalar(
                        out=h_, in0=h_, scalar1=-0.5, scalar2=1.5,
                        op0=ALU.mult, op1=ALU.add)
                    nc.vector.tensor_tensor(out=y, in0=y, in1=h_, op=ALU.mult)
                nc.vector.tensor_copy(out=rstd, in_=y)
                nmr = sA.tile([128, 4, 4], F32, name="nmr")
                nc.vector.tensor_tensor(out=nmr, in0=mvp[:, :, :, 0],
                                        in1=rstd, op=ALU.mult)
                nc.vector.tensor_scalar_mul(out=nmr, in0=nmr, scalar1=-1.0)

                for gi, (rt_, sb_) in enumerate(held):
                    norm = []
                    for si, (off, w) in enumerate(segs):
                        if si < 3:
                            dst = sA.tile([128, 128], BF16, name=f"n{si}")
                        else:
                            dst = v_sb[:, rt_, 0:DV]
                        if si < 2:
                            nc.vector.tensor_scalar(
                                out=dst, in0=sb_[:, off:off + w],
                                scalar1=mvp[:, gi, si, 0:1],
                                scalar2=rstd[:, gi, si:si + 1],
                                op0=ALU.subtract, op1=ALU.mult)
                        else:
                            nc.scalar.activation(
                                out=dst, in_=sb_[:, off:off + w],
                                func=AF.Identity,
                                scale=rstd[:, gi, si:si + 1],
                                bias=nmr[:, gi, si:si + 1])
                        if si < 3:
                            norm.append(dst)
                    pending_tr.append((rt_, norm[0], norm[1], norm[2]))

            def emit_rt(rt):
                xc = rt // RT_PER_XC
                xoff = (rt % RT_PER_XC) * 128
                if xc not in _xt_chunks:
                    load_xt(xc)
                xt_sb = _xt_chunks[xc]
                # ready transposes fill the PE while the qkv psum ring waits
                # on the previous row tile's evacuation
                for _ in range(2):
                    if len(pending_tr) >= 2:
                        emit_tr(*pending_tr.pop(0))

                # 576 psum columns span 2 banks -> two matmul targets
                qkv_a = psW.tile([128, 512], F32, name="qkv_a", tag="sim", bufs=2)
                qkv_b = psW.tile([128, 512], F32, name="qkv_b", tag="w", bufs=2)[:, :64]
                for dt_ in range(DIMT):
                    nc.tensor.matmul(
                        qkv_a, lhsT=xt_sb[:, dt_, xoff:xoff + 128],
                        rhs=w_all_sb[:, dt_, 0:512],
                        start=(dt_ == 0), stop=(dt_ == DIMT - 1))
                    nc.tensor.matmul(
                        qkv_b, lhsT=xt_sb[:, dt_, xoff:xoff + 128],
                        rhs=w_all_sb[:, dt_, 512:WCOLS],
                        start=(dt_ == 0), stop=(dt_ == DIMT - 1))
                qkv_sb = sA.tile([128, WCOLS], F32, name="qkv_sb", tag="qkv",
                                 bufs=5)
                nc.scalar.activation(out=qkv_sb[:, 0:512], in_=qkv_a,
                                     func=AF.Copy)
                nc.scalar.activation(out=qkv_sb[:, 512:WCOLS], in_=qkv_b,
                                     func=AF.Copy)

                g = rt % 4
                if g == 0:
                    _grp["mvp"] = sA.tile([128, 4, 4, 2], F32, name="mvp")
                mvp = _grp["mvp"]
                stats = sA.tile([128, 4, 6], F32, name="stats")
                for si, (off, w) in enumerate(segs):
                    nc.vector.bn_stats(out=stats[:, si, :], in_=qkv_sb[:, off:off + w])
                    nc.vector.bn_aggr(out=mvp[:, g, si, :], in_=stats[:, si, :])
                _grp["held"].append((rt, qkv_sb))
                if g == 3:
                    finalize_group()

            def flush_tr():
                while pending_tr:
                    emit_tr(*pending_tr.pop(0))

            # phase 1: row tiles 0-7 stand alone (needed before any attention);
            # row tiles 8-15 are injected into the first attention kt-loop.
            # DMA order: w half-1 (above), x chunk 0, w half-2 -- the first
            # qkv matmuls gate on w.h1 + xt0.q1, not the whole head
            load_xt(0)
            nc.sync.dma_start(out=w_all_sb[:, DTH:, :], in_=w_all_r[:, DTH:, :])
            nc.sync.dma_start(out=gq_sb, in_=gq[:, :])
            nc.sync.dma_start(out=gk_sb, in_=gk[:, :])
            for rt in range(8):
                emit_rt(rt)
            flush_tr()
            load_xt(2, engine=nc.scalar)
            load_xt(3, engine=nc.scalar)
            inject_rts = list(range(8, RT))

            # ---------------- stage B: attention + output projection --------
            # w_out loaded here so the head DMA queue serves stage A first.
            w_out_t = []
            for h in range(HPC):
                wa = const.tile([128, dim], BF16)
                nc.scalar.dma_start(out=wa, in_=w_out[h * DV:h * DV + 128, :])
                wb = const.tile([64, dim], BF16)
                nc.scalar.dma_start(out=wb, in_=w_out[h * DV + 128:(h + 1) * DV, :])
                w_out_t.append((wa, wb))

            import functools

            pending_po = []

            def emit_po(outUa_, outUb_, rcol_, h_, qoff_, t, cc, tail=False):
                po = psW.tile([128, 512], F32, name="po", tag="w", bufs=2)
                wa, wb = w_out_t[h_]
                nc.tensor.matmul(
                    po, lhsT=outUa_[:, t * 128:(t + 1) * 128],
                    rhs=wa[:, cc * 512:(cc + 1) * 512],
                    start=True, stop=False)
                nc.tensor.matmul(
                    po, lhsT=outUb_[:, t * 128:(t + 1) * 128],
                    rhs=wb[:, cc * 512:(cc + 1) * 512],
                    start=False, stop=True)
                po_sb = sB.tile([128, 512], BF16, name="po_sb")
                if (t * CC + cc) % 2 == 1:
                    nc.scalar.activation(out=po_sb, in_=po, func=AF.Copy,
                                         scale=rcol_[:, t:t + 1])
                else:
                    nc.vector.tensor_scalar_mul(
                        out=po_sb, in0=po, scalar1=rcol_[:, t:t + 1])
                eng = nc.scalar if (tail and (t * CC + cc) % 2 == 0) else nc.sync
                eng.dma_start(
                    out=out[h_, qoff_ + t * 128: qoff_ + (t + 1) * 128,
                            cc * 512:(cc + 1) * 512],
                    in_=po_sb)

            for h in range(HPC):
                for qh in range(QH):
                    qoff = qh * QHW
                    accA = [psAcc.tile([128, 512], F32, name=f"accA{qc}",
                                       tag=f"accA{qc}")
                            for qc in range(QC)]
                    accB = [psAcc.tile([65, 512], F32, name=f"accB{qc}",
                                       tag=f"accB{qc}")
                            for qc in range(QC)]

                    def fire_av(pkt, e1):
                        # e1: [128, QHW] exp row for key tile pkt
                        for qc in range(QC):
                            nc.tensor.matmul(
                                accA[qc], lhsT=v_sb[:, pkt, 0:128],
                                rhs=e1[:, qc * 512:(qc + 1) * 512],
                                start=(pkt == 0), stop=(pkt == KT - 1))
                            nc.tensor.matmul(
                                accB[qc], lhsT=v_sb[:, pkt, 128:DV + 1],
                                rhs=e1[:, qc * 512:(qc + 1) * 512],
                                start=(pkt == 0), stop=(pkt == KT - 1))

                    pending_av = []
                    th4 = None
                    for kt in range(KT):
                        # tanh/exp group sizes [2,2,4,4,2,1,1]: short groups
                        # at the pass edges cut the sim->tanh->exp->av bubble
                        if kt < 4 or kt in (12, 13):
                            gsz, gslot = 2, kt % 2
                        elif kt >= 14:
                            gsz, gslot = 1, 0
                        else:
                            gsz, gslot = 4, (kt - 4) % 4
                        if gslot == 0:
                            th4 = sTh.tile([128, gsz, QHW], BF16, name="th4")
                        bias_sb = sBias.tile([128, QHW], BF16, name="bias_sb")
                        nc.sync.dma_start(
                            out=bias_sb,
                            in_=biasT[h, kt * 128:(kt + 1) * 128,
                                      qoff:qoff + QHW])

                        # --- older, dependency-free work first so the PE
                        # FIFO never head-of-line blocks on this kt's sim ---
                        nfire = (min(3, len(pending_av)) if kt >= KT - 2
                                 else (2 if len(pending_av) >= 3 else 0))
                        for _ in range(nfire):
                            fire_av(*pending_av.pop(0))

                        if kt >= 3:
                            for _ in range(2):
                                if pending_po:
                                    pending_po.pop(0)()

                        if h == 0 and qh == 0:
                            if inject_rts:
                                emit_rt(inject_rts.pop(0))
                            elif pending_tr:
                                # transposes of injected rts must land before
                                # their kT columns are consumed in THIS loop
                                emit_tr(*pending_tr.pop(0))
                                if kt >= 10:
                                    flush_tr()

                        for qc in range(QC):
                            sim_ps = psW.tile([128, 512], F32, name="sim_ps",
                                              tag="sim", bufs=2)
                            nc.tensor.matmul(
                                sim_ps,
                                lhsT=kT_sb[:, kt * 128:(kt + 1) * 128],
                                rhs=qT_sb[h][:, qoff + qc * 512:
                                             qoff + (qc + 1) * 512],
                                start=True, stop=True)
                            # bias add rides the PSUM->SBUF evacuation (DVE)
                            nc.vector.tensor_tensor(
                                out=th4[:, gslot, qc * 512:(qc + 1) * 512],
                                in0=sim_ps,
                                in1=bias_sb[:, qc * 512:(qc + 1) * 512],
                                op=ALU.add)
                        if gslot == gsz - 1:
                            t4 = sT4.tile([128, gsz, QHW], F32, name="t4",
                                          bufs=2)
                            nc.scalar.activation(out=t4, in_=th4, func=AF.Tanh,
                                                 scale=1.0 / CLAMP)
                            e4 = sE.tile([128, gsz, QHW], BF16, name="e4")
                            nc.scalar.activation(out=e4, in_=t4, func=AF.Exp,
                                                 scale=CLAMP)
                            for j in range(gsz):
                                pending_av.append((kt - gsz + 1 + j,
                                                   e4[:, j, :]))

                    for pkt, e1 in pending_av:
                        fire_av(pkt, e1)

                    def make_evac(accA_, accB_, h_, qoff_):
                        def evac():
                            outUa = sB.tile([128, QHW], BF16, name="outUa")
                            outUb = sB.tile([64, QHW], BF16, name="outUb")
                            s_row = sB.tile([1, QHW], F32, name="s_row")
                            for qc in range(QC):
                                nc.vector.tensor_copy(
                                    out=s_row[:, qc * 512:(qc + 1) * 512],
                                    in_=accB_[qc][64:65, :])
                            s_dram = sDram.tile([1, QHW], F32, name="s_dram")
                            nc.sync.dma_start(out=s_dram, in_=s_row)
                            for qc in range(QC):
                                # ACT copies: they land in ACT's idle window
                                # between the last exp and the next pass tanh
                                nc.scalar.activation(
                                    out=outUa[:, qc * 512:(qc + 1) * 512],
                                    in_=accA_[qc], func=AF.Copy)
                                nc.scalar.activation(
                                    out=outUb[:, qc * 512:(qc + 1) * 512],
                                    in_=accB_[qc][0:64, :], func=AF.Copy)
                            rcol_raw = sB.tile([128, QHW // 128], F32,
                                               name="rcol_raw")
                            nc.sync.dma_start(
                                out=rcol_raw,
                                in_=s_dram.rearrange("one (t p) -> p (one t)",
                                                     p=128))
                            rcol = sB.tile([128, QHW // 128], F32, name="rcol")
                            nc.vector.reciprocal(out=rcol, in_=rcol_raw)
                            for t in range(QHW // 128):
                                for cc in range(CC):
                                    pending_po.append(functools.partial(
                                        emit_po, outUa, outUb, rcol,
                                        h_, qoff_, t, cc))
                        return evac

                    make_evac(accA, accB, h, qoff)()

                if h == 0:
                    flush_tr()

            for fn in pending_po:
                fn(tail=True)

    nc.compile()
    return nc


_NC_CACHE = {}


def _get_nc(n=N, dim=DIM):
    key = (n, dim)
    if key not in _NC_CACHE:
        _NC_CACHE[key] = build_nc(n, dim)
    return _NC_CACHE[key]


def make_in_maps(x, attn_bias, w_qkv, w_out, g_q, g_k, g_v, n_cores=N_CORES):
    """Host-side shard + preprocess. Returns per-core input maps."""
    b, n, dim = x.shape
    bf = ml_dtypes.bfloat16
    xT_b = [np.ascontiguousarray(x[bb].T).astype(bf) for bb in range(b)]
    kv_cols = w_qkv[:, HEADS * DQK:].astype(np.float32)      # [dim, 320]
    w_out_g = (w_out.astype(np.float32)
               * np.tile(g_v.astype(np.float32), HEADS)[:, None])
    in_maps = []
    for c in range(n_cores):
        bb = c % 2
        h0 = (c // 2) * HPC
        w_q = [w_qkv[:, (h0 + i) * DQK:(h0 + i + 1) * DQK] for i in range(HPC)]
        w_all = np.concatenate(w_q + [kv_cols], axis=1).astype(bf)  # [dim, 576]
        biasT = np.ascontiguousarray(
            attn_bias[bb, h0:h0 + HPC].transpose(0, 2, 1)).astype(bf)
        w_out_h = w_out_g[h0 * DV:(h0 + HPC) * DV, :].astype(bf)   # [384, dim]
        in_maps.append({
            "xT": xT_b[bb],
            "w_all": w_all,
            "biasT": biasT,
            "w_out": w_out_h,
            "gq": (g_q * SCALE).astype(np.float32).reshape(DQK, 1),
            "gk": g_k.astype(np.float32).reshape(DQK, 1),
        })
    return in_maps


def kernel(x, attn_bias, w_qkv, w_out, g_q, g_k, g_v):
    x = np.asarray(x, dtype=np.float32)
    attn_bias = np.asarray(attn_bias, dtype=np.float32)
    w_qkv = np.asarray(w_qkv, dtype=np.float32)
    w_out = np.asarray(w_out, dtype=np.float32)
    g_q = np.asarray(g_q, dtype=np.float32)
    g_k = np.asarray(g_k, dtype=np.float32)
    g_v = np.asarray(g_v, dtype=np.float32)

    b, n, dim = x.shape
    nc = _get_nc(n, dim)
    in_maps = make_in_maps(x, attn_bias, w_qkv, w_out, g_q, g_k, g_v)
    res = run_bass_kernel_spmd(nc, in_maps, core_ids=list(range(N_CORES)),
                               trace=bool(os.environ.get("KERNEL_TRACE")))
    _LAST_STATS["exec_time_ns"] = res.exec_time_ns
    _LAST_STATS["mean_exec_time_ns"] = res.mean_exec_time_ns
    _LAST_STATS["res"] = res
    out = np.zeros((b, n, dim), dtype=np.float32)
    for c in range(N_CORES):
        part = res.results[c]["out"].astype(np.float32)
        out[c % 2] += part[0] + part[1]
    return out


# revision 22
# speedup vs baseline: 1.0750x; 1.0504x over previous
"""Trainium2 Bass kernel for multi-query attention with tanh-clamped softmax.

Sharding: each core takes ONE batch and TWO query heads (8 cores = 2 batches x
4 head-pairs). K/V projections are shared (multi-query) so they are replicated
across the 4 cores of a batch; Q columns and W_out rows are tensor-parallel by
head. Each core emits per-head partial outputs [2, n, dim]; the host sums the
8 partial planes per batch (row-parallel unshard).

Versus the 1-head x 2-batch layout this halves the projected token count
(stage A: 72->46us PE) and halves the number of host partials.

All matmuls run in bf16 with fp32 PSUM accumulation. Key scheduling choices:
- The attn bias is added on the DVE during the sim PSUM->SBUF evacuation
  (tensor_tensor add), NOT via an identity matmul on the PE: saves ~27us of
  PE time, and the SBUF staging lets tanh/exp run as wide [128,4096] ACT
  instructions (4 key-tiles per instr), cutting ACT fixed overheads.
- LN rsqrt runs as a DVE-only Newton iteration everywhere (the ACT Sqrt
  would thrash ~1.3us activation-table reloads against tanh/exp).
- Row tiles 8-15 of the QKV projection are injected into the first
  attention pass's kt-loop so the PE queue stays dense; their x chunks are
  prefetched on the second HWDGE queue.
- Softmax row-sums ride a ones-column in the attn@v accumulation; their
  reciprocal is reshaped to per-partition columns via a DRAM bounce and
  folded into the per-head output-projection evacuation as a per-token scale.
- ~3.5us of dummy matmuls pre-warm the PE HAM clock gate during the DMA head.
"""

import os
import sys

sys.path.insert(0, "/opt/trn_rl_repo")

import numpy as np
import ml_dtypes

import concourse.bass as bass
import concourse.tile as tile
from concourse import bacc, mybir
from concourse.bass_utils import run_bass_kernel_spmd
from concourse.masks import make_identity

F32 = mybir.dt.float32
BF16 = mybir.dt.bfloat16
AF = mybir.ActivationFunctionType
ALU = mybir.AluOpType

HEADS = 8
DQK = 128
DV = 192
SCALE = 64 ** -0.5
CLAMP = 5.0
EPS = 1e-5

B = 2
N = 2048
DIM = 1536
N_CORES = 8
HPC = 2                      # heads per core

_LAST_STATS = {}


def _enable_ldw_opt():
    """The container's baked compiler flags carry --enable-ldw-opt=false;
    flip just that option for this kernel's compile (it removes the
    per-matmul serialized LDWEIGHTS)."""
    try:
        from concourse.compiler_utils import get_compiler_flags, set_compiler_flags

        flags = [f.replace("--enable-ldw-opt=false", "--enable-ldw-opt=true")
                 for f in get_compiler_flags()]
        set_compiler_flags(flags)
    except Exception:
        pass


def build_nc(n=N, dim=DIM):
    """Build the per-core Bass graph. All cores run the same graph (SPMD)."""
    _enable_ldw_opt()
    assert dim % 512 == 0 and n % 1024 == 0
    DIMT = dim // 128          # 12 contraction tiles for projections
    RT = n // 128              # 16 row tiles (one batch)
    KT = n // 128              # 16 key tiles
    QH = n // 1024             # 2 q-half passes
    QHW = 1024
    QC = QHW // 512            # 2
    CC = dim // 512            # 3 output column chunks
    WCOLS = HPC * DQK + DQK + DV   # 576

    nc = bacc.Bacc("TRN2", target_bir_lowering=False)

    xT = nc.declare_dram_parameter("xT", [dim, n], BF16, isOutput=False)
    w_all = nc.declare_dram_parameter("w_all", [dim, WCOLS], BF16, isOutput=False)
    biasT = nc.declare_dram_parameter("biasT", [HPC, n, n], BF16, isOutput=False)
    w_out = nc.declare_dram_parameter("w_out", [HPC * DV, dim], BF16, isOutput=False)
    gq = nc.declare_dram_parameter("gq", [DQK, 1], F32, isOutput=False)
    gk = nc.declare_dram_parameter("gk", [DQK, 1], F32, isOutput=False)
    out = nc.declare_dram_parameter("out", [HPC, n, dim], BF16, isOutput=True)

    with tile.TileContext(nc) as tc:
        with (
            tc.tile_pool(name="const", bufs=1) as const,
            tc.tile_pool(name="big", bufs=1) as big,
            tc.tile_pool(name="stA", bufs=5) as sA,
            tc.tile_pool(name="stB", bufs=3) as sB,
            tc.tile_pool(name="biasp", bufs=6) as sBias,
            tc.tile_pool(name="thp", bufs=2) as sTh,
            tc.tile_pool(name="t4p", bufs=1) as sT4,
            tc.tile_pool(name="expp", bufs=2) as sE,
            tc.tile_pool(name="dramp", bufs=3, space="DRAM") as sDram,
            tc.tile_pool(name="work_ps", bufs=4, space="PSUM") as psW,
            tc.tile_pool(name="acc_ps", bufs=1, space="PSUM") as psAcc,
        ):
            # ---------------- constants ----------------
            DTH = DIMT // 2
            w_all_sb = const.tile([128, DIMT, WCOLS], BF16)
            w_all_r = w_all.rearrange("(t p) c -> p t c", p=128)
            nc.sync.dma_start(out=w_all_sb[:, :DTH, :], in_=w_all_r[:, :DTH, :])
            gq_sb = const.tile([128, 1], F32)
            gk_sb = const.tile([128, 1], F32)
            ident = const.tile([128, 128], BF16)
            make_identity(nc, ident)
            I32 = mybir.dt.int32
            magic_sb = const.tile([128, 4, 4], I32)
            nc.vector.memset(magic_sb, 0x5F3759DF)
            # HAM pre-warm: dummy PE activity during the DMA head so the
            # first real matmuls start at the full clock
            warm_sb = const.tile([128, 128], BF16)
            nc.vector.memset(warm_sb, 0.0)
            warm_ps = psW.tile([128, 512], F32, name="warm_ps", tag="w", bufs=2)
            for _ in range(64):
                nc.tensor.matmul(warm_ps[:, :128], lhsT=warm_sb, rhs=warm_sb,
                                 start=True, stop=True)

            # ---------------- resident activations ----------------
            NXC = 4
            XCW = n // NXC                 # 512 tokens per x chunk
            RT_PER_XC = XCW // 128         # 4 row tiles per chunk
            xTr = xT.rearrange("(t p) r -> p t r", p=128)

            qT_sb = [big.tile([128, n], BF16, name=f"qT{h}") for h in range(HPC)]
            kT_sb = big.tile([128, n], BF16, name="kT")
            v_sb = big.tile([128, KT, 208], BF16, name="v")
            nc.vector.memset(v_sb[:, :, DV:DV + 1], 1.0)

            # ---------------- stage A: QKV projection + LN + transpose ----------
            pending_tr = []

            def emit_tr(ktile_, qn0_, qn1_, kn_):
                dsts = [(qn0_, qT_sb[0], gq_sb), (qn1_, qT_sb[1], gq_sb),
                        (kn_, kT_sb, gk_sb)]
                for src, dstt, g in dsts:
                    tp = psW.tile([128, 512], BF16, name="tp", tag="w", bufs=2)[:, :128]
                    nc.tensor.transpose(tp, src, ident)
                    nc.vector.tensor_scalar_mul(
                        out=dstt[:, ktile_ * 128:(ktile_ + 1) * 128],
                        in0=tp, scalar1=g)

            _grp = {"mvp": None, "held": []}
            _xt_chunks = {}

            def load_xt(xc, engine=None):
                eng = engine or nc.sync
                xt_sb = sA.tile([128, DIMT, XCW], BF16, name="xt_sb", tag="xt",
                                bufs=2)
                nsplit = 4 if xc == 0 else 2
                step = DIMT // nsplit
                for s in range(nsplit):
                    eng.dma_start(
                        out=xt_sb[:, s * step:(s + 1) * step, :],
                        in_=xTr[:, s * step:(s + 1) * step,
                                xc * XCW:(xc + 1) * XCW],
                    )
                _xt_chunks[xc] = xt_sb

            # segments of the qkv projection output: q_h0, q_h1, k, v
            segs = [(0, DQK), (DQK, DQK), (2 * DQK, DQK), (3 * DQK, DV)]

            def finalize_group():
                # one Newton rsqrt chain for FOUR row tiles: the chain is
                # ~20 serial DVE ops, so batching amortizes its latency
                mvp = _grp["mvp"]
                held = _grp["held"]
                _grp["held"] = []
                rstd = sA.tile([128, 4, 4], F32, name="rstd")
                xv = sA.tile([128, 4, 4], F32, name="rsq_x")
                nc.vector.tensor_scalar_add(out=xv, in0=mvp[:, :, :, 1],
                                            scalar1=EPS)
                yi = sA.tile([128, 4, 4], I32, name="rsq_yi")
                nc.vector.tensor_scalar(
                    out=yi, in0=xv.bitcast(I32), scalar1=1, scalar2=None,
                    op0=ALU.logical_shift_right)
                nc.vector.tensor_tensor(out=yi, in0=magic_sb, in1=yi,
                                        op=ALU.subtract)
                y = yi.bitcast(F32)
                h_ = sA.tile([128, 4, 4], F32, name="rsq_h")
                for _ in range(3):
                    nc.vector.tensor_tensor(out=h_, in0=y, in1=y, op=ALU.mult)
                    nc.vector.tensor_tensor(out=h_, in0=h_, in1=xv, op=ALU.mult)
                    nc.vector.tensor_scalar(
                        out=h_, in0=h_, scalar1=-0.5, scalar2=1.5,
                        op0=ALU.mult, op1=ALU.add)
                    nc.vector.tensor_tensor(out=y, in0=y, in1=h_, op=ALU.mult)
                nc.vector.tensor_copy(out=rstd, in_=y)
                nmr = sA.tile([128, 4, 4], F32, name="nmr")
                nc.vector.tensor_tensor(out=nmr, in0=mvp[:, :, :, 0],
                                        in1=rstd, op=ALU.mult)
                nc.vector.tensor_scalar_mul(out=nmr, in0=nmr, scalar1=-1.0)

                for gi, (rt_, sb_) in enumerate(held):
                    norm = []
                    for si, (off, w) in enumerate(segs):
                        if si < 3:
                            dst = sA.tile([128, 128], BF16, name=f"n{si}")
                        else:
                            dst = v_sb[:, rt_, 0:DV]
                        if si < 2:
                            nc.vector.tensor_scalar(
                                out=dst, in0=sb_[:, off:off + w],
                                scalar1=mvp[:, gi, si, 0:1],
                                scalar2=rstd[:, gi, si:si + 1],
                                op0=ALU.subtract, op1=ALU.mult)
                        else:
                            nc.scalar.activation(
                                out=dst, in_=sb_[:, off:off + w],
                                func=AF.Identity,
                                scale=rstd[:, gi, si:si + 1],
                                bias=nmr[:, gi, si:si + 1])
                        if si < 3:
                            norm.append(dst)
                    pending_tr.append((rt_, norm[0], norm[1], norm[2]))

            def emit_rt(rt):
                xc = rt // RT_PER_XC
                xoff = (rt % RT_PER_XC) * 128
                if xc not in _xt_chunks:
                    load_xt(xc)
                xt_sb = _xt_chunks[xc]
                # ready transposes fill the PE while the qkv psum ring waits
                # on the previous row tile's evacuation
                for _ in range(2):
                    if len(pending_tr) >= 2:
                        emit_tr(*pending_tr.pop(0))

                # 576 psum columns span 2 banks -> two matmul targets
                qkv_a = psW.tile([128, 512], F32, name="qkv_a", tag="w", bufs=2)
                qkv_b = psW.tile([128, 512], F32, name="qkv_b", tag="w", bufs=2)[:, :64]
                for dt_ in range(DIMT):
                    nc.tensor.matmul(
                        qkv_a, lhsT=xt_sb[:, dt_, xoff:xoff + 128],
                        rhs=w_all_sb[:, dt_, 0:512],
                        start=(dt_ == 0), stop=(dt_ == DIMT - 1))
                    nc.tensor.matmul(
                        qkv_b, lhsT=xt_sb[:, dt_, xoff:xoff + 128],
                        rhs=w_all_sb[:, dt_, 512:WCOLS],
                        start=(dt_ == 0), stop=(dt_ == DIMT - 1))
                qkv_sb = sA.tile([128, WCOLS], F32, name="qkv_sb", tag="qkv",
                                 bufs=5)
                nc.scalar.activation(out=qkv_sb[:, 0:512], in_=qkv_a,
                                     func=AF.Copy)
                nc.scalar.activation(out=qkv_sb[:, 512:WCOLS], in_=qkv_b,
                                     func=AF.Copy)

                g = rt % 4
                if g == 0:
                    _grp["mvp"] = sA.tile([128, 4, 4, 2], F32, name="mvp")
                mvp = _grp["mvp"]
                stats = sA.tile([128, 4, 6], F32, name="stats")
                for si, (off, w) in enumerate(segs):
                    nc.vector.bn_stats(out=stats[:, si, :], in_=qkv_sb[:, off:off + w])
                    nc.vector.bn_aggr(out=mvp[:, g, si, :], in_=stats[:, si, :])
                _grp["held"].append((rt, qkv_sb))
                if g == 3:
                    finalize_group()

            def flush_tr():
                while pending_tr:
                    emit_tr(*pending_tr.pop(0))

            # phase 1: row tiles 0-7 stand alone (needed before any attention);
            # row tiles 8-15 are injected into the first attention kt-loop.
            # DMA order: w half-1 (above), x chunk 0, w half-2 -- the first
            # qkv matmuls gate on w.h1 + xt0.q1, not the whole head
            load_xt(0)
            nc.sync.dma_start(out=w_all_sb[:, DTH:, :], in_=w_all_r[:, DTH:, :])
            nc.sync.dma_start(out=gq_sb, in_=gq[:, :])
            nc.sync.dma_start(out=gk_sb, in_=gk[:, :])
            for rt in range(8):
                emit_rt(rt)
            flush_tr()
            load_xt(2, engine=nc.scalar)
            load_xt(3, engine=nc.scalar)
            inject_rts = list(range(8, RT))

            # ---------------- stage B: attention + output projection --------
            # w_out loaded here so the head DMA queue serves stage A first.
            w_out_t = []
            for h in range(HPC):
                wa = const.tile([128, dim], BF16)
                nc.scalar.dma_start(out=wa, in_=w_out[h * DV:h * DV + 128, :])
                wb = const.tile([64, dim], BF16)
                nc.scalar.dma_start(out=wb, in_=w_out[h * DV + 128:(h + 1) * DV, :])
                w_out_t.append((wa, wb))

            import functools

            pending_po = []

            def emit_po(outUa_, outUb_, rcol_, h_, qoff_, t, cc, tail=False):
                po = psW.tile([128, 512], F32, name="po", tag="w", bufs=2)
                wa, wb = w_out_t[h_]
                nc.tensor.matmul(
                    po, lhsT=outUa_[:, t * 128:(t + 1) * 128],
                    rhs=wa[:, cc * 512:(cc + 1) * 512],
                    start=True, stop=False)
                nc.tensor.matmul(
                    po, lhsT=outUb_[:, t * 128:(t + 1) * 128],
                    rhs=wb[:, cc * 512:(cc + 1) * 512],
                    start=False, stop=True)
                po_sb = sB.tile([128, 512], BF16, name="po_sb")
                if (t * CC + cc) % 2 == 1:
                    nc.scalar.activation(out=po_sb, in_=po, func=AF.Copy,
                                         scale=rcol_[:, t:t + 1])
                else:
                    nc.vector.tensor_scalar_mul(
                        out=po_sb, in0=po, scalar1=rcol_[:, t:t + 1])
                eng = nc.scalar if (tail and (t * CC + cc) % 2 == 0) else nc.sync
                eng.dma_start(
                    out=out[h_, qoff_ + t * 128: qoff_ + (t + 1) * 128,
                            cc * 512:(cc + 1) * 512],
                    in_=po_sb)

            for h in range(HPC):
                for qh in range(QH):
                    qoff = qh * QHW
                    accA = [psAcc.tile([128, 512], F32, name=f"accA{qc}",
                                       tag=f"accA{qc}")
                            for qc in range(QC)]
                    accB = [psAcc.tile([65, 512], F32, name=f"accB{qc}",
                                       tag=f"accB{qc}")
                            for qc in range(QC)]

                    def fire_av(pkt, e1):
                        # e1: [128, QHW] exp row for key tile pkt
                        for qc in range(QC):
                            nc.tensor.matmul(
                                accA[qc], lhsT=v_sb[:, pkt, 0:128],
                                rhs=e1[:, qc * 512:(qc + 1) * 512],
                                start=(pkt == 0), stop=(pkt == KT - 1))
                            nc.tensor.matmul(
                                accB[qc], lhsT=v_sb[:, pkt, 128:DV + 1],
                                rhs=e1[:, qc * 512:(qc + 1) * 512],
                                start=(pkt == 0), stop=(pkt == KT - 1))

                    pending_av = []
                    th4 = None
                    for kt in range(KT):
                        # tanh/exp group sizes [2,2,4,4,2,1,1]: short groups
                        # at the pass edges cut the sim->tanh->exp->av bubble
                        if kt < 4 or kt in (12, 13):
                            gsz, gslot = 2, kt % 2
                        elif kt >= 14:
                            gsz, gslot = 1, 0
                        else:
                            gsz, gslot = 4, (kt - 4) % 4
                        if gslot == 0:
                            th4 = sTh.tile([128, gsz, QHW], BF16, name="th4")
                        bias_sb = sBias.tile([128, QHW], BF16, name="bias_sb")
                        nc.sync.dma_start(
                            out=bias_sb,
                            in_=biasT[h, kt * 128:(kt + 1) * 128,
                                      qoff:qoff + QHW])

                        # --- older, dependency-free work first so the PE
                        # FIFO never head-of-line blocks on this kt's sim ---
                        nfire = (min(3, len(pending_av)) if kt >= KT - 2
                                 else (2 if len(pending_av) >= 3 else 0))
                        for _ in range(nfire):
                            fire_av(*pending_av.pop(0))

                        if kt >= 3:
                            for _ in range(2):
                                if pending_po:
                                    pending_po.pop(0)()

                        if h == 0 and qh == 0:
                            if inject_rts:
                                emit_rt(inject_rts.pop(0))
                            elif pending_tr:
                                # transposes of injected rts must land before
                                # their kT columns are consumed in THIS loop
                                emit_tr(*pending_tr.pop(0))
                                if kt >= 10:
                                    flush_tr()

                        for qc in range(QC):
                            sim_ps = psW.tile([128, 512], F32, name="sim_ps",
                                              tag="sim", bufs=2)
                            nc.tensor.matmul(
                                sim_ps,
                                lhsT=kT_sb[:, kt * 128:(kt + 1) * 128],
                                rhs=qT_sb[h][:, qoff + qc * 512:
                                             qoff + (qc + 1) * 512],
                                start=True, stop=True)
                            # bias add rides the PSUM->SBUF evacuation (DVE)
                            nc.vector.tensor_tensor(
                                out=th4[:, gslot, qc * 512:(qc + 1) * 512],
                                in0=sim_ps,
                                in1=bias_sb[:, qc * 512:(qc + 1) * 512],
                                op=ALU.add)
                        if gslot == gsz - 1:
                            t4 = sT4.tile([128, gsz, QHW], F32, name="t4",
                                          bufs=2)
                            nc.scalar.activation(out=t4, in_=th4, func=AF.Tanh,
                                                 scale=1.0 / CLAMP)
                            e4 = sE.tile([128, gsz, QHW], BF16, name="e4")
                            nc.scalar.activation(out=e4, in_=t4, func=AF.Exp,
                                                 scale=CLAMP)
                            for j in range(gsz):
                                pending_av.append((kt - gsz + 1 + j,
                                                   e4[:, j, :]))

                    for pkt, e1 in pending_av:
                        fire_av(pkt, e1)

                    def make_evac(accA_, accB_, h_, qoff_):
                        def evac():
                            outUa = sB.tile([128, QHW], BF16, name="outUa")
                            outUb = sB.tile([64, QHW], BF16, name="outUb")
                            s_row = sB.tile([1, QHW], F32, name="s_row")
                            for qc in range(QC):
                                nc.vector.tensor_copy(
                                    out=s_row[:, qc * 512:(qc + 1) * 512],
                                    in_=accB_[qc][64:65, :])
                            s_dram = sDram.tile([1, QHW], F32, name="s_dram")
                            nc.sync.dma_start(out=s_dram, in_=s_row)
                            for qc in range(QC):
                                # ACT copies: they land in ACT's idle window
                                # between the last exp and the next pass tanh
                                nc.scalar.activation(
                                    out=outUa[:, qc * 512:(qc + 1) * 512],
                                    in_=accA_[qc], func=AF.Copy)
                                nc.scalar.activation(
                                    out=outUb[:, qc * 512:(qc + 1) * 512],
                                    in_=accB_[qc][0:64, :], func=AF.Copy)
                            rcol_raw = sB.tile([128, QHW // 128], F32,
                                               name="rcol_raw")
                            nc.sync.dma_start(
                                out=rcol_raw,
                                in_=s_dram.rearrange("one (t p) -> p (one t)",
                                                     p=128))
                            rcol = sB.tile([128, QHW // 128], F32, name="rcol")
                            nc.vector.reciprocal(out=rcol, in_=rcol_raw)
                            for t in range(QHW // 128):
                                for cc in range(CC):
                                    pending_po.append(functools.partial(
                                        emit_po, outUa, outUb, rcol,
                                        h_, qoff_, t, cc))
                        return evac

                    make_evac(accA, accB, h, qoff)()

                if h == 0:
                    flush_tr()

            for fn in pending_po:
                fn(tail=True)

    nc.compile()
    return nc


_NC_CACHE = {}


def _get_nc(n=N, dim=DIM):
    key = (n, dim)
    if key not in _NC_CACHE:
        _NC_CACHE[key] = build_nc(n, dim)
    return _NC_CACHE[key]


def make_in_maps(x, attn_bias, w_qkv, w_out, g_q, g_k, g_v, n_cores=N_CORES):
    """Host-side shard + preprocess. Returns per-core input maps."""
    b, n, dim = x.shape
    bf = ml_dtypes.bfloat16
    xT_b = [np.ascontiguousarray(x[bb].T).astype(bf) for bb in range(b)]
    kv_cols = w_qkv[:, HEADS * DQK:].astype(np.float32)      # [dim, 320]
    w_out_g = (w_out.astype(np.float32)
               * np.tile(g_v.astype(np.float32), HEADS)[:, None])
    in_maps = []
    for c in range(n_cores):
        bb = c % 2
        h0 = (c // 2) * HPC
        w_q = [w_qkv[:, (h0 + i) * DQK:(h0 + i + 1) * DQK] for i in range(HPC)]
        w_all = np.concatenate(w_q + [kv_cols], axis=1).astype(bf)  # [dim, 576]
        biasT = np.ascontiguousarray(
            attn_bias[bb, h0:h0 + HPC].transpose(0, 2, 1)).astype(bf)
        w_out_h = w_out_g[h0 * DV:(h0 + HPC) * DV, :].astype(bf)   # [384, dim]
        in_maps.append({
            "xT": xT_b[bb],
            "w_all": w_all,
            "biasT": biasT,
            "w_out": w_out_h,
            "gq": (g_q * SCALE).astype(np.float32).reshape(DQK, 1),
            "gk": g_k.astype(np.float32).reshape(DQK, 1),
        })
    return in_maps


def kernel(x, attn_bias, w_qkv, w_out, g_q, g_k, g_v):
    x = np.asarray(x, dtype=np.float32)
    attn_bias = np.asarray(attn_bias, dtype=np.float32)
    w_qkv = np.asarray(w_qkv, dtype=np.float32)
    w_out = np.asarray(w_out, dtype=np.float32)
    g_q = np.asarray(g_q, dtype=np.float32)
    g_k = np.asarray(g_k, dtype=np.float32)
    g_v = np.asarray(g_v, dtype=np.float32)

    b, n, dim = x.shape
    nc = _get_nc(n, dim)
    in_maps = make_in_maps(x, attn_bias, w_qkv, w_out, g_q, g_k, g_v)
    res = run_bass_kernel_spmd(nc, in_maps, core_ids=list(range(N_CORES)),
                               trace=bool(os.environ.get("KERNEL_TRACE")))
    _LAST_STATS["exec_time_ns"] = res.exec_time_ns
    _LAST_STATS["mean_exec_time_ns"] = res.mean_exec_time_ns
    _LAST_STATS["res"] = res
    out = np.zeros((b, n, dim), dtype=np.float32)
    for c in range(N_CORES):
        part = res.results[c]["out"].astype(np.float32)
        out[c % 2] += part[0] + part[1]
    return out
